# revision 7
# baseline (speedup 1.0000x reference)
import sys, os
sys.path.insert(0, '/opt/trn_rl_repo')
import numpy as np

import concourse.bass as bass
import concourse.bacc as bacc
import concourse.tile as tile
from concourse import mybir
from concourse import bass_utils

# ---- problem constants (hardcoded; kernel.py must be self-contained) ----
B, N = 8, 2048
DIN, DH, DE = 32, 64, 64
R = 3
NT = N // 128         # 16 blocks of 128 nodes
EPS_BN = 1e-5
EPS_NORM = 1e-12
F16 = mybir.dt.float16
F32 = mybir.dt.float32
I32 = mybir.dt.int32

_cached = {}


def _build():
    nc = bacc.Bacc("TRN2", target_bir_lowering=False, debug=False,
                   enable_asserts=False, num_devices=8)

    rel = nc.dram_tensor("rel", [N, N], I32, kind="ExternalInput")
    x16 = nc.dram_tensor("x16", [N, DIN], F16, kind="ExternalInput")
    wcat = nc.dram_tensor("wcat", [R * DIN + 1, DH], F16, kind="ExternalInput")
    w2b = nc.dram_tensor("w2b", [DH + 1, DH], F16, kind="ExternalInput")
    w3b = nc.dram_tensor("w3b", [DH + 1, DE], F16, kind="ExternalInput")
    m1_out = nc.dram_tensor("m1", [128, DH], F16, kind="ExternalOutput")
    m2_out = nc.dram_tensor("m2", [128, DH], F16, kind="ExternalOutput")
    m3_out = nc.dram_tensor("m3", [128, DE], F16, kind="ExternalOutput")

    with tile.TileContext(nc) as tc:
        _kern(nc, tc, rel, x16, wcat, w2b, w3b, m1_out, m2_out, m3_out)
    nc.compile()
    return nc


def _kern(nc, tc, rel, x16, wcat, w2b, w3b, m1_out, m2_out, m3_out):
    from contextlib import ExitStack
    ctx = ExitStack()
    with ctx:
        const = ctx.enter_context(tc.tile_pool(name="const", bufs=1))
        persist = ctx.enter_context(tc.tile_pool(name="persist", bufs=1))
        rpool = ctx.enter_context(tc.tile_pool(name="rp", bufs=3))
        tpool = ctx.enter_context(tc.tile_pool(name="tp", bufs=3))
        mpool = ctx.enter_context(tc.tile_pool(name="mp", bufs=2))
        spool = ctx.enter_context(tc.tile_pool(name="sp", bufs=4))
        dram = ctx.enter_context(tc.tile_pool(name="dram", bufs=1, space="DRAM"))

        # constants / persistent tensors
        x_sb = const.tile([128, NT, DIN], F16)           # x[128k+p, d] at [p,k,d]
        nc.sync.dma_start(x_sb[:], x16.ap().rearrange("(k p) d -> p k d", p=128))
        wcat_sb = const.tile([R * DIN + 1, DH], F16)
        nc.sync.dma_start(wcat_sb[:], wcat.ap())
        w2b_sb = const.tile([DH + 1, DH], F16)
        nc.sync.dma_start(w2b_sb[:], w2b.ap())
        w3b_sb = const.tile([DH + 1, DE], F16)
        nc.sync.dma_start(w3b_sb[:], w3b.ap())

        AT = persist.tile([128, NT, N], F16)             # adjT[j,n] at [j%128, j//128, n]
        hT = persist.tile([R * DIN + 1, N], F16)         # [h1T;h2T;h3T;ones]
        h2T = persist.tile([DH + 1, N], F16)
        h3T = persist.tile([DH + 1, N], F16)
        u1 = persist.tile([128, NT, DH], F32)            # relu(norm(y1)) pre-BN
        u2 = persist.tile([128, NT, DH], F32)
        v1 = persist.tile([128, NT, DH], F16)            # BN output (L2 lhsT)
        v2 = persist.tile([128, NT, DH], F16)
        stats = persist.tile([128, 2 * NT], F32)         # S1 cols 0:16, S2 cols 16:32
        gstats = persist.tile([128, 2 * NT], F32)
        mean = persist.tile([128, NT], F32)
        rs = persist.tile([128, NT], F32)
        M1 = persist.tile([128, DH], F16)
        M2 = persist.tile([128, DH], F16)
        M3 = persist.tile([128, DE], F16)
        scr = persist.tile([128, DH], F32)               # epilogue scratch
        nrm = persist.tile([128, 4], F32)

        nc.vector.memset(hT[R * DIN:R * DIN + 1, :], 1.0)
        nc.vector.memset(h2T[DH:DH + 1, :], 1.0)
        nc.vector.memset(h3T[DH:DH + 1, :], 1.0)

        # ---------------- Phase 1: load rel, transpose, masks, L1 aggregation
        l1ctx = ExitStack()
        l1ps = l1ctx.enter_context(tc.tile_pool(name="l1ps", bufs=2, space="PSUM"))
        for c in range(NT):
            rc = rpool.tile([128, N], F16, tag="rc")
            nc.gpsimd.dma_start(rc[:], rel.ap()[128 * c:128 * (c + 1), :])  # cast i32->f16
            tcb = tpool.tile([128, NT, 128], F16, tag="tc")
            nc.sync.dma_start_transpose(tcb[:], rc[:])   # tcb[p,k,n'] = rel[128c+n', 128k+p]
            nc.vector.tensor_scalar(AT[:, :, 128 * c:128 * (c + 1)], tcb[:], 0.0, None,
                                    mybir.AluOpType.is_gt)
            masks = []
            for i in range(R):
                mi = mpool.tile([128, NT, 128], F16, tag=f"m{i}")
                nc.vector.tensor_scalar(mi[:], tcb[:], float(i + 1), None,
                                        mybir.AluOpType.is_equal)
                masks.append(mi)
            hps = [l1ps.tile([DIN, 128], F32, tag=f"hp{i}", name=f"hp{i}_{c}")
                   for i in range(R)]
            for k in range(NT):
                for i in range(R):
                    nc.tensor.matmul(hps[i][:], x_sb[:, k, :],
                                     masks[i][:, k, :],
                                     start=(k == 0), stop=(k == NT - 1),
                                     skip_group_check=True)
            for i in range(R):
                nc.vector.tensor_copy(
                    hT[DIN * i:DIN * (i + 1), 128 * c:128 * (c + 1)], hps[i][:])
        l1ctx.close()

        # ---------------- layer epilogue helper
        def proj_epilogue(hT_sb, w_sb, kdim, u_sb, Mx, layer3, pp):
            for t in range(NT):
                yp = pp.tile([128, DH], F32, tag="yp")
                nc.tensor.matmul(yp[:], hT_sb[0:kdim + 1, 128 * t:128 * (t + 1)],
                                 w_sb[:], start=True, stop=True)
                # L2 norm over free dim
                nc.scalar.square(scr[:], yp[:])
                nc.vector.reduce_sum(nrm[:, 0:1], scr[:], axis=mybir.AxisListType.X)
                nc.scalar.sqrt(nrm[:, 1:2], nrm[:, 0:1])
                nc.vector.tensor_scalar_max(nrm[:, 2:3], nrm[:, 1:2], EPS_NORM)
                nc.vector.reciprocal(nrm[:, 3:4], nrm[:, 2:3])
                if layer3:
                    # z = y * rn ; running max in M3
                    nc.vector.tensor_scalar(scr[:], yp[:], nrm[:, 3:4], None,
                                            mybir.AluOpType.mult)
                    if t == 0:
                        nc.vector.tensor_copy(Mx[:], scr[:])
                    else:
                        nc.vector.tensor_tensor(Mx[:], Mx[:], scr[:],
                                                mybir.AluOpType.max)
                else:
                    # u = relu(y * rn); stats
                    nc.vector.tensor_scalar(u_sb[:, t, :], yp[:], nrm[:, 3:4], 0.0,
                                            mybir.AluOpType.mult, mybir.AluOpType.max)
                    nc.vector.reduce_sum(stats[:, t:t + 1], u_sb[:, t, :],
                                         axis=mybir.AxisListType.X)
                    nc.scalar.square(scr[:], u_sb[:, t, :])
                    nc.vector.reduce_sum(stats[:, NT + t:NT + t + 1], scr[:],
                                         axis=mybir.AxisListType.X)

        def bn_allreduce_apply(u_sb, v_sb, Mx):
            bin_ = dram.tile([128, 2 * NT], F32, tag="bin")
            bout = dram.tile([128, 2 * NT], F32, tag="bout")
            nc.gpsimd.dma_start(bin_[:], stats[:])
            nc.gpsimd.collective_compute(
                "AllReduce", mybir.AluOpType.add,
                replica_groups=[list(range(8))],
                ins=[bin_.opt()], outs=[bout.opt()])
            nc.gpsimd.dma_start(gstats[:], bout[:])
            cnt = 1.0 / (B * DH)
            nc.vector.tensor_scalar(mean[:], gstats[:, 0:NT], cnt, None,
                                    mybir.AluOpType.mult)
            nc.scalar.square(rs[:], mean[:])            # rs = mean^2 (scratch)
            nc.vector.tensor_scalar(gstats[:, NT:2 * NT], gstats[:, NT:2 * NT],
                                    cnt, EPS_BN,
                                    mybir.AluOpType.mult, mybir.AluOpType.add)
            nc.vector.tensor_tensor(rs[:], gstats[:, NT:2 * NT], rs[:],
                                    mybir.AluOpType.subtract)   # var + eps
            nc.scalar.sqrt(rs[:], rs[:])
            nc.vector.reciprocal(rs[:], rs[:])
            for t in range(NT):
                nc.vector.tensor_scalar(v_sb[:, t, :], u_sb[:, t, :],
                                        mean[:, t:t + 1], rs[:, t:t + 1],
                                        mybir.AluOpType.subtract,
                                        mybir.AluOpType.mult)
                if t == 0:
                    nc.vector.tensor_copy(Mx[:], v_sb[:, t, :])
                else:
                    nc.vector.tensor_tensor(Mx[:], Mx[:], v_sb[:, t, :],
                                            mybir.AluOpType.max)

        def agg(v_sb, outT, ps):
            hp2 = ps.tile([DH, N], F32, tag="hp2")
            for cg in range(4):
                for k in range(NT):
                    nc.tensor.matmul(hp2[:, 512 * cg:512 * (cg + 1)], v_sb[:, k, :],
                                     AT[:, k, 512 * cg:512 * (cg + 1)],
                                     start=(k == 0), stop=(k == NT - 1),
                                     skip_group_check=True)
            nc.vector.tensor_copy(outT[0:DH, :], hp2[:])

        pp = ctx.enter_context(tc.tile_pool(name="pp", bufs=2, space="PSUM"))
        proj_epilogue(hT, wcat_sb, R * DIN, u1, M1, False, pp)
        bn_allreduce_apply(u1, v1, M1)

        aggps = ctx.enter_context(tc.tile_pool(name="aggps", bufs=1, space="PSUM"))
        agg(v1, h2T, aggps)
        proj_epilogue(h2T, w2b_sb, DH, u2, M2, False, pp)
        bn_allreduce_apply(u2, v2, M2)

        agg(v2, h3T, aggps)
        proj_epilogue(h3T, w3b_sb, DH, None, M3, True, pp)

        nc.sync.dma_start(m1_out.ap(), M1[:])
        nc.sync.dma_start(m2_out.ap(), M2[:])
        nc.sync.dma_start(m3_out.ap(), M3[:])


def _get_nc():
    if "nc" not in _cached:
        _cached["nc"] = _build()
    return _cached["nc"]


_last_exec_ns = [None]


def kernel(x, relation, adj, w_first, b_first, w_block, b_block,
           w_last, b_last, w_map, b_map):
    x = np.asarray(x); relation = np.asarray(relation)
    w_first = np.asarray(w_first); b_first = np.asarray(b_first)
    w_block = np.asarray(w_block); b_block = np.asarray(b_block)
    w_last = np.asarray(w_last); b_last = np.asarray(b_last)
    w_map = np.asarray(w_map); b_map = np.asarray(b_map)

    wcat = np.concatenate([w_first.reshape(R * DIN, DH), b_first[None, :]], 0).astype(np.float16)
    w2b = np.concatenate([w_block[0], b_block[None, :]], 0).astype(np.float16)
    w3b = np.concatenate([w_last[0], b_last[None, :]], 0).astype(np.float16)

    nc = _get_nc()
    in_maps = []
    for c in range(B):
        in_maps.append({
            "rel": np.ascontiguousarray(relation[c]).astype(np.int32),
            "x16": x[c].astype(np.float16),
            "wcat": wcat, "w2b": w2b, "w3b": w3b,
        })
    trace = os.environ.get("KTRACE", "0") == "1"
    res = bass_utils.run_bass_kernel_spmd(nc, in_maps, core_ids=list(range(8)),
                                          trace=trace)
    _last_exec_ns[0] = res.exec_time_ns

    output = np.zeros((B, 3 * DH), np.float32)
    for c in range(B):
        r = res.results[c]
        output[c, 0:DH] = r["m1"].astype(np.float32).max(0)
        output[c, DH:2 * DH] = r["m2"].astype(np.float32).max(0)
        output[c, 2 * DH:3 * DH] = r["m3"].astype(np.float32).max(0)
    ypred = output @ w_map.astype(np.float32) + b_map.astype(np.float32)
    return (output, ypred)
